# revision 16
# baseline (speedup 1.0000x reference)
"""Causal self-attention on 8 TRN2 NeuronCores.

Sharding: head-parallel x batch-parallel. Core (b, g) computes heads
[8g, 8g+8) of batch b: qkv projection for those heads, causal attention,
and a partial output projection (rows 512g..512g+512 of W_proj). The host
sums the two partials per batch and adds b_proj.

v3 design (fp16 operands, fp32 accumulation):
- Feature-major layout throughout (no on-device transposes): x^T comes in
  pre-transposed; Q^T/K^T are produced feature-major; V token-major with
  per-head augmented columns so PV row-sums and partition placement fall
  out of the PV matmul itself.
- Scores S^T = K Q^T per head pair are computed K=64-packed: the two heads
  of a pair live on partitions 0:64 / 64:128 of qt/kt, so their score
  matmuls target different PE row-groups and can run concurrently (2x).
  Issue order alternates heads to maximize overlap.
- Softmax: exp on ACT (no max subtraction; scores are ~N(0,1)), triangle
  mask on diagonal 128-blocks on DVE, row sums via an augmented ones
  column in V. Even heads' PV lands at psum partitions 0:65 (sums at 64);
  odd heads use a 128-wide augmented V [dead32|ones|dead31|v] so sums land
  at partition 32 and y at 64:128 - all partition-aligned. yps psum is
  freed immediately by one DVE copy to SBUF; reciprocal+broadcast+scale
  then run off the psum critical path, using only the base-partition-0
  gpsimd broadcast pattern that is known to work on hardware.
- Pipelining: V proj, then QK for pair 0, then per pair: attention with
  the next pair's QK projection (or, for the last pair, the output
  projection chunks) interleaved into the tensor-engine slack.
- PSUM budget (8 banks): scores 2x[128,1024] (4) + yps even/odd (2) +
  aux for interleaved QK/proj (2).
"""

import sys

sys.path.insert(0, "/opt/trn_rl_repo")

import numpy as np

import concourse.bass as bass  # noqa: F401
import concourse.tile as tile
from concourse import bacc, mybir
from concourse.bass_utils import run_bass_kernel_spmd

B, T, C = 4, 2048, 1024
H, D = 16, 64
G = 2           # head groups (cores per batch)
HPG = H // G    # heads per core = 8
PAIRS = HPG // 2
CH = T // 512   # 4 q-chunks of 512
ST = T // 128   # 16 s-tiles of 128
KO = C // 128   # 8 contraction tiles
# per-pair augmented V width: [v0|ones] (65) + [dead32|ones|dead31|v1] (128).
# The odd head's 128-wide stationary makes its PV output span psum partitions
# 0:128 (matmul psum writes must start at partition 0): row sums land at
# partition 32, y at 64:128 - every later consumer is partition-aligned.
VW = 193

F32 = mybir.dt.float32
F16 = mybir.dt.float16
EXP = mybir.ActivationFunctionType.Exp

_CACHED_NC = None


def _build(repeat=1, phases='ABC', pss_bufs=1, psy_bufs=1,
           aux_bufs=2, psa_bufs=4, pb_bufs=6, pr_bufs=3, pc_bufs=6,
           mask_pool=False, interleave=True):
    nc = bacc.Bacc("TRN2", target_bir_lowering=False, debug=False)
    xT = nc.dram_tensor("xT", [C, T], F16, kind="ExternalInput").ap()
    wqk = nc.dram_tensor("wqk", [PAIRS, 128, KO, 2, 128], F16,
                         kind="ExternalInput").ap()
    wv = nc.dram_tensor("wv", [128, KO, 512], F16, kind="ExternalInput").ap()
    wp = nc.dram_tensor("wp", [128, PAIRS, 8, 128], F16,
                        kind="ExternalInput").ap()
    bqk = nc.dram_tensor("bqk", [128, 2, PAIRS], F32, kind="ExternalInput").ap()
    bv = nc.dram_tensor("bv", [1, 512], F32, kind="ExternalInput").ap()
    mask = nc.dram_tensor("mask", [128, 128], F16, kind="ExternalInput").ap()
    vcst = nc.dram_tensor("vcst", [128, PAIRS, 65], F16,
                          kind="ExternalInput").ap()
    out = nc.dram_tensor("out", [C, T], F16, kind="ExternalOutput").ap()

    mask_eng = "gpsimd" if mask_pool else "vector"

    with tile.TileContext(nc) as tc:
      for _rep in range(repeat):
        with tc.tile_pool(name="persist", bufs=1) as pp:
            v_sb = [pp.tile([128, PAIRS, VW], F16, name=f"v{i}", tag=f"v{i}")
                    for i in range(ST)]
            qt = [pp.tile([128, T], F16, name=f"qt{p}", tag=f"q{p}")
                  for p in range(PAIRS)]
            kt = [pp.tile([128, T], F16, name=f"kt{p}", tag=f"k{p}")
                  for p in range(PAIRS)]
            y2tp = [pp.tile([128, T], F16, name=f"y2t{p}", tag=f"y{p}")
                    for p in range(PAIRS)]
            xt_sb = [pp.tile([128, T], F16, name=f"xt{ko}", tag=f"xt{ko}")
                     for ko in range(KO)]
            wqk_sb = pp.tile([128, PAIRS, KO, 2, 128], F16)
            wp_sb = pp.tile([128, PAIRS, 8, 128], F16)
            mask_sb = pp.tile([128, 128], F16)
            bqk_sb = pp.tile([128, 2, PAIRS], F32)
            bv_bc = pp.tile([128, 512], F32)
            wv_sb = pp.tile([128, KO, 512], F16)

            # DMA issue order = earliest consumer first (the SP HWDGE ring is
            # FIFO): V needs wv + xT chunk 0; wqk is consumed after V; mask at
            # phase-B start; wp only at the last pair's projection.
            def xchunk(j):
                for ko in range(KO):
                    nc.sync.dma_start(
                        xt_sb[ko][:, 512 * j:512 * j + 512],
                        xT[128 * ko:128 * ko + 128, 512 * j:512 * j + 512])

            nc.sync.dma_start(wv_sb[:], wv)
            nc.sync.dma_start(bqk_sb[:], bqk)
            with tc.tile_pool(name="ph0", bufs=1) as p0:
                bv_sb = p0.tile([1, 512], F32)
                nc.sync.dma_start(bv_sb[:], bv)
                nc.gpsimd.partition_broadcast(bv_bc[:], bv_sb[0:1, :])
            xchunk(0)
            # v_sb aug columns [64:129]: even ones at 64, odd ones at 97,
            # dead zeros elsewhere - from a host constant
            for i in range(ST):
                nc.sync.dma_start(v_sb[i][:, :, 64:129], vcst)
            xchunk(1)
            xchunk(2)
            nc.sync.dma_start(wqk_sb[:], wqk.rearrange("p k o t m -> k p o t m"))
            xchunk(3)
            nc.sync.dma_start(mask_sb[:], mask)
            nc.sync.dma_start(wp_sb[:], wp)

            with tc.tile_pool(name="psA", bufs=psa_bufs, space="PSUM") as psa:
                # ---- V (token-major, all 8 heads) ----
                for si in range(ST):
                    ps = psa.tile([128, 512], F32, tag="psA")
                    for ko in range(KO):
                        nc.tensor.matmul(
                            ps[:], xt_sb[ko][:, 128 * si:128 * si + 128],
                            wv_sb[:, ko, :],
                            start=(ko == 0), stop=(ko == KO - 1))
                    # scatter into per-pair augmented slots (+ bias)
                    pv = ps[:].rearrange("s (p two d) -> s p two d", two=2, d=64)
                    bvv = bv_bc[:].rearrange("s (p two d) -> s p two d",
                                             two=2, d=64)
                    nc.vector.tensor_add(
                        v_sb[si][:, :, 0:64], pv[:, :, 0, :], bvv[:, :, 0, :])
                    nc.vector.tensor_add(
                        v_sb[si][:, :, 129:193], pv[:, :, 1, :], bvv[:, :, 1, :])

                def qk_chunk(p, j, pool):
                    for t, dst in ((0, qt[p]), (1, kt[p])):
                        ps = pool.tile([128, 512], F32, tag=pool.name)
                        for ko in range(KO):
                            nc.tensor.matmul(
                                ps[:], wqk_sb[:, p, ko, t, :],
                                xt_sb[ko][:, 512 * j:512 * j + 512],
                                start=(ko == 0), stop=(ko == KO - 1))
                        nc.vector.tensor_scalar_add(
                            dst[:, 512 * j:512 * j + 512], ps[:],
                            bqk_sb[:, t, p:p + 1])

                # ---- QK for pair 0 (rest are interleaved into phase B) ----
                for j in range(CH):
                    qk_chunk(0, j, psa)
                if not interleave or 'B' not in phases:
                    for p in range(1, PAIRS):
                        for j in range(CH):
                            qk_chunk(p, j, psa)

            # ---------------- phase B: attention (+ interleaved QK / proj) --
            if 'B' not in phases:
                continue
            with tc.tile_pool(name="phB_p", bufs=pb_bufs) as pb, \
                 tc.tile_pool(name="phB_r", bufs=pr_bufs) as pr, \
                 tc.tile_pool(name="phC", bufs=pc_bufs) as pc, \
                 tc.tile_pool(name="psS", bufs=pss_bufs, space="PSUM") as pss, \
                 tc.tile_pool(name="psYe", bufs=psy_bufs, space="PSUM") as pse, \
                 tc.tile_pool(name="psYo", bufs=psy_bufs, space="PSUM") as pso, \
                 tc.tile_pool(name="aux", bufs=aux_bufs, space="PSUM") as paux:

                def proj_chunk(j):
                    for u in proj_units(j):
                        u()

                def proj_units(j):
                    """Decomposed proj chunk: one thunk per PE/DVE/DMA step so
                    the work can be spread into attention's PE slack."""
                    units = []
                    for o in range(8):
                        st = {}

                        def mk_mm(o, p2, st=None):
                            def f(st=st, o=o, p2=p2):
                                if p2 == 0:
                                    st["ps"] = paux.tile([128, 512], F32,
                                                         tag="aux", name="psc")
                                nc.tensor.matmul(
                                    st["ps"][:], wp_sb[:, p2, o, :],
                                    y2tp[p2][:, 512 * j:512 * (j + 1)],
                                    start=(p2 == 0), stop=(p2 == PAIRS - 1))
                            return f

                        def mk_out(o, st=st):
                            def f():
                                ob = pc.tile([128, 512], F16, tag="ob",
                                             name="ob")
                                nc.vector.tensor_copy(ob[:], st["ps"][:])
                                nc.sync.dma_start(
                                    out[128 * o:128 * o + 128,
                                        512 * j:512 * (j + 1)], ob[:])
                            return f

                        for p2 in range(PAIRS):
                            units.append(mk_mm(o, p2, st))
                        units.append(mk_out(o))
                    return units

                def qk_units(p, j):
                    units = []
                    for t, dst in ((0, qt[p]), (1, kt[p])):
                        st = {}

                        def mk_mm(t, ko, st=st):
                            def f():
                                if ko == 0:
                                    st["ps"] = paux.tile([128, 512], F32,
                                                         tag="aux", name="psq")
                                nc.tensor.matmul(
                                    st["ps"][:], wqk_sb[:, p, ko, t, :],
                                    xt_sb[ko][:, 512 * j:512 * j + 512],
                                    start=(ko == 0), stop=(ko == KO - 1))
                            return f

                        def mk_add(t, dst, st=st):
                            def f():
                                nc.vector.tensor_scalar_add(
                                    dst[:, 512 * j:512 * j + 512], st["ps"][:],
                                    bqk_sb[:, t, p:p + 1])
                            return f

                        for ko in range(KO):
                            units.append(mk_mm(t, ko, st))
                        units.append(mk_add(t, dst))
                    return units

                for p in range(PAIRS):
                    for j in range(CH):
                        n_tiles = 4 * j + 4
                        n_iters = n_tiles // 2
                        ye = pse.tile([128, 512], F32, name="ye", tag="Ye")
                        yo = pso.tile([128, 512], F32, name="yo", tag="Yo")

                        # PE filler for the exp-wait bubbles: next pair's QK
                        # projection, or (last pair) the previous chunk's
                        # output projection.
                        filler = []
                        if interleave:
                            if p < PAIRS - 1:
                                filler = qk_units(p + 1, j)
                            elif 'C' in phases and j > 0:
                                filler = proj_units(j - 1)
                        fi = 0

                        def emit_pv(ptp, g0, offs):
                            for h in range(2):
                                for u in range(2):
                                    i = g0 + u
                                    vsl = (v_sb[i][:, p, 0:65] if h == 0 else
                                           v_sb[i][:, p, 65:193])
                                    yp = (ye[0:65] if h == 0 else yo[0:128])
                                    nc.tensor.matmul(
                                        yp[:, offs[u]:512],
                                        vsl,
                                        ptp[:, 1024 * h + 512 * u + offs[u]:
                                            1024 * h + 512 * (u + 1)],
                                        start=(i == 0),
                                        stop=(i == n_tiles - 1))

                        # software pipeline: scores+exp for iter n, then PV
                        # for iter n-1, so the PE never waits on the current
                        # exp. Score matmuls always write full 512-col halves
                        # (exp must not read unwritten psum); causal trimming
                        # happens in the PV reads and the mask.
                        prev = None
                        for it, g0 in enumerate(range(0, n_tiles, 2)):
                            spair = pss.tile([128, 2048], F32, name="spair",
                                             tag="S")
                            ptp = pb.tile([128, 2048], F16, name="ptp", tag="P")
                            offs = [max(0, 128 * (g0 + u) - 512 * j)
                                    for u in range(2)]
                            # alternate heads so the two PE row-groups
                            # (partitions 0:64 / 64:128) overlap
                            for u in range(2):
                                i = g0 + u
                                for h in range(2):
                                    lo, hi = 64 * h, 64 * h + 64
                                    nc.tensor.matmul(
                                        spair[:, 1024 * h + 512 * u:
                                              1024 * h + 512 * (u + 1)],
                                        kt[p][lo:hi, 128 * i:128 * i + 128],
                                        qt[p][lo:hi,
                                              512 * j:512 * (j + 1)],
                                        start=True, stop=True)
                            nc.scalar.activation(
                                ptp[:, offs[0]:2048],
                                spair[:, offs[0]:2048], EXP)
                            for h in range(2):
                                for u in range(2):
                                    i = g0 + u
                                    if i >= 4 * j:  # diagonal 128-block
                                        dlo = 1024 * h + 512 * u + offs[u]
                                        nc.vector.tensor_mul(
                                            ptp[:, dlo:dlo + 128],
                                            ptp[:, dlo:dlo + 128],
                                            mask_sb[:])
                            # spread filler over iterations
                            take = (len(filler) - fi + n_iters - it - 1) \
                                // (n_iters - it)
                            for _ in range(take):
                                filler[fi]()
                                fi += 1
                            if prev is not None:
                                emit_pv(*prev)
                            prev = (ptp, g0, offs)
                        while fi < len(filler):
                            filler[fi]()
                            fi += 1
                        emit_pv(*prev)

                        # free psum fast: one DVE copy each (y+sums together),
                        # then normalize off the psum path. Row sums sit at
                        # partition 64 (even) / 32 (odd); y at 0:64 / 64:128.
                        yre = pr.tile([128, 512], F16, tag="yre", name="yre")
                        yro = pr.tile([128, 512], F16, tag="yro", name="yro")
                        nc.vector.tensor_copy(yre[0:65, :], ye[0:65, :])
                        nc.vector.tensor_copy(yro[:, :], yo[:, :])
                        rr = pr.tile([128, 1024], F16, tag="rr", name="rr")
                        r0 = pr.tile([1, 1024], F16, tag="r0", name="r0")
                        rb = pr.tile([128, 1024], F16, tag="rb", name="rb")
                        with nc.allow_low_precision(reason="softmax recip fp16"):
                            nc.vector.reciprocal(rr[64:65, 0:512],
                                                 yre[64:65, :])
                            nc.vector.reciprocal(rr[32:33, 512:1024],
                                                 yro[32:33, :])
                        nc.sync.dma_start(r0[0:1, 0:512], rr[64:65, 0:512])
                        nc.sync.dma_start(r0[0:1, 512:1024], rr[32:33, 512:1024])
                        nc.gpsimd.partition_broadcast(rb[:], r0[0:1, :])
                        nc.vector.tensor_mul(
                            y2tp[p][0:64, 512 * j:512 * (j + 1)],
                            yre[0:64, :], rb[0:64, 0:512])
                        nc.vector.tensor_mul(
                            y2tp[p][64:128, 512 * j:512 * (j + 1)],
                            yro[64:128, :], rb[64:128, 512:1024])

                if interleave and 'C' in phases:
                    proj_chunk(CH - 1)
                if not interleave and 'C' in phases:
                    for j in range(CH):
                        proj_chunk(j)
    nc.compile()
    return nc


def _get_nc():
    global _CACHED_NC
    if _CACHED_NC is None:
        _CACHED_NC = _build()
    return _CACHED_NC


def null_io_spec():
    """External I/O of _build()'s module, for bench.build_null()."""
    return [
        ("xT", [C, T], F16, "ExternalInput"),
        ("wqk", [PAIRS, 128, KO, 2, 128], F16, "ExternalInput"),
        ("wv", [128, KO, 512], F16, "ExternalInput"),
        ("wp", [128, PAIRS, 8, 128], F16, "ExternalInput"),
        ("bqk", [128, 2, PAIRS], F32, "ExternalInput"),
        ("bv", [1, 512], F32, "ExternalInput"),
        ("mask", [128, 128], F16, "ExternalInput"),
        ("vcst", [128, PAIRS, 65], F16, "ExternalInput"),
        ("out", [C, T], F16, "ExternalOutput"),
    ]


def _prep_in_maps(x, W_qkv, b_qkv, W_proj, b_proj):
    x = np.asarray(x, dtype=np.float32)
    W_qkv = np.asarray(W_qkv, dtype=np.float32)
    b_qkv = np.asarray(b_qkv, dtype=np.float32)
    W_proj = np.asarray(W_proj, dtype=np.float32)
    scale = np.float32(1.0 / np.sqrt(D))
    mask = np.triu(np.ones((128, 128), dtype=np.float16))
    vcst = np.zeros((128, PAIRS, 65), dtype=np.float16)
    vcst[:, :, 0] = 1.0    # even-head ones column (v_sb col 64)
    vcst[:, :, 33] = 1.0   # odd-head ones column (v_sb col 97)

    per_g = []
    for g in range(G):
        cs, ce = 512 * g, 512 * g + 512
        Wq = W_qkv[:, cs:ce] * scale
        Wk = W_qkv[:, C + cs:C + ce]
        Wv = W_qkv[:, 2 * C + cs:2 * C + ce]
        # wqk[p, ki, ko, t, m] = W_t[128*ko + ki, 128*p + m]
        qk = np.stack([Wq, Wk], axis=0)  # (2, C, 512)
        qk = qk.reshape(2, KO, 128, PAIRS, 128)
        wqk_h = np.ascontiguousarray(
            qk.transpose(3, 2, 1, 0, 4)).astype(np.float16)
        wv_b = np.ascontiguousarray(
            Wv.reshape(KO, 128, 512).transpose(1, 0, 2)).astype(np.float16)
        # wp[ki, p, o, m] = W_proj[512*g + 128*p + ki, 128*o + m]
        wp_b = np.ascontiguousarray(
            W_proj[cs:ce].reshape(PAIRS, 128, 8, 128)
            .transpose(1, 0, 2, 3)).astype(np.float16)
        bq = b_qkv[cs:ce] * scale
        bk = b_qkv[C + cs:C + ce]
        # bqk[ki, t, p] = b_t[128*p + ki]
        bqk_b = np.ascontiguousarray(
            np.stack([bq, bk], 0).reshape(2, PAIRS, 128).transpose(2, 0, 1))
        bv_b = np.ascontiguousarray(
            b_qkv[2 * C + cs:2 * C + ce].reshape(1, 512))
        per_g.append(dict(wqk=wqk_h, wv=wv_b, wp=wp_b, bqk=bqk_b, bv=bv_b,
                          mask=mask, vcst=vcst))

    in_maps = []
    for b in range(B):
        xTb = np.ascontiguousarray(x[b].T).astype(np.float16)
        for g in range(G):
            in_maps.append({"xT": xTb, **per_g[g]})
    return in_maps


def kernel(x, W_qkv, b_qkv, W_proj, b_proj):
    nc = _get_nc()
    in_maps = _prep_in_maps(x, W_qkv, b_qkv, W_proj, b_proj)
    res = run_bass_kernel_spmd(nc, in_maps, core_ids=list(range(8)))
    b_proj = np.asarray(b_proj, dtype=np.float32)
    out = np.empty((B, T, C), dtype=np.float32)
    for b in range(B):
        acc = (res.results[2 * b]["out"].astype(np.float32)
               + res.results[2 * b + 1]["out"].astype(np.float32))
        out[b] = acc.T + b_proj
    return out
